# revision 10
# baseline (speedup 1.0000x reference)
"""Trainium2 Bass kernel for nn_AttentionRouting.

Reference computation (per sample):
  pooled = mean(embedding, spatial)            [G=8, CIN=64]
  h      = relu(w1[g] @ pooled[g] + b1[g])     [G, 512]
  atts   = w2[g] @ h[g] + b2[g]                [G, 256]
  routed = 3-iter dynamic routing over xr=atts.reshape(G, CAPS=4, OUT=64)
  out    = sigmoid(routed)[ch] * x[:, ch]      (per-channel scale of x)

Sharding: pure data parallel over batch (B=32 -> 4 samples per core x 8 cores).
Weights replicated. Everything below is hardcoded to those shapes.
"""

import numpy as np

import concourse.bass as bass
import concourse.bacc as bacc
import concourse.mybir as mybir
import concourse.tile as tile
from concourse.bass_utils import run_bass_kernel_spmd

F32 = mybir.dt.float32
AF = mybir.ActivationFunctionType
AX = mybir.AxisListType

N_CORES = 8
B_LOC = 4            # samples per core
G = 8                # groups
CIN = 64             # channels per group (embedding)
HID = 512            # hidden dim of the squeeze MLP
CAPS = 4
OUT = 64
NCH = CAPS * OUT     # 256 x-channels
HW = 64 * 64         # 4096 spatial
ITERS = 3

EMB_ROWS = B_LOC * G * CIN     # 2048
X_ROWS = B_LOC * NCH           # 1024
EMB_TILES = EMB_ROWS // 128    # 16  (row blocks of 128 channels)
X_TILES = X_ROWS // 128        # 8
TW = HW // 2                   # 2048-wide half tiles (1 MB)


def _consts():
    i128 = np.eye(128, dtype=np.float32)
    i4 = np.eye(4, dtype=np.float32)
    # sel32to4[g*4+b, b] = 1  (sums over g when used as lhsT)
    sel = np.zeros((32, 4), dtype=np.float32)
    sel[np.arange(32), np.arange(32) % 4] = 1.0
    # sel4to32[b, g*4+b] = 1  (broadcasts over g when used as lhsT)
    selt = np.ascontiguousarray(sel.T)
    return i128, i4, sel, selt


def build_nc():
    nc = bacc.Bacc()
    emb = nc.dram_tensor("emb", [EMB_ROWS, HW], F32, kind="ExternalInput")
    xin = nc.dram_tensor("xin", [X_ROWS, HW], F32, kind="ExternalInput")
    # host-prepared weight layouts (see kernel() below)
    w1t = nc.dram_tensor("w1t", [CIN, G * HID], F32, kind="ExternalInput")
    b1t = nc.dram_tensor("b1t", [128, G * 4], F32, kind="ExternalInput")
    w2t = nc.dram_tensor("w2t", [128, G * 4 * NCH], F32, kind="ExternalInput")
    b2t = nc.dram_tensor("b2t", [128, G * 2], F32, kind="ExternalInput")
    out = nc.dram_tensor("out", [X_ROWS, HW], F32, kind="ExternalOutput")

    i128_np, i4_np, sel_np, selt_np = _consts()
    i128_d = nc.inline_tensor(i128_np, "ident128")
    i4_d = nc.inline_tensor(i4_np, "ident4")
    sel_d = nc.inline_tensor(sel_np, "sel32to4")
    selt_d = nc.inline_tensor(selt_np, "sel4to32")

    with tile.TileContext(nc) as tc:
        with (
            tc.tile_pool(name="consts", bufs=1) as cp,
            tc.tile_pool(name="stats", bufs=1) as sp,
            tc.tile_pool(name="embp", bufs=8) as embp,
            tc.tile_pool(name="xp", bufs=8) as xp,
            tc.tile_pool(name="psA", bufs=2, space="PSUM") as psA,
            tc.tile_pool(name="psB", bufs=4, space="PSUM") as psB,
        ):
            # ---- load constants / weights into SBUF -------------------
            w1t_sb = cp.tile([CIN, G * HID], F32, tag="w1t")
            b1t_sb = cp.tile([128, G * 4], F32, tag="b1t")
            w2t_sb = cp.tile([128, G * 4 * NCH], F32, tag="w2t")
            b2t_sb = cp.tile([128, G * 2], F32, tag="b2t")
            i128_sb = cp.tile([128, 128], F32, tag="i128")
            i4_sb = cp.tile([4, 4], F32, tag="i4")
            sel_sb = cp.tile([32, 4], F32, tag="sel")
            selt_sb = cp.tile([4, 32], F32, tag="selt")
            nc.sync.dma_start(w1t_sb[:], w1t[:])
            nc.sync.dma_start(b1t_sb[:], b1t[:])
            nc.sync.dma_start(w2t_sb[:], w2t[:])
            nc.sync.dma_start(b2t_sb[:], b2t[:])
            nc.sync.dma_start(i128_sb[:], i128_d[:])
            nc.sync.dma_start(i4_sb[:], i4_d[:])
            nc.sync.dma_start(sel_sb[:], sel_d[:])
            nc.sync.dma_start(selt_sb[:], selt_d[:])

            # ---- phase 1: stream embedding, spatial sums --------------
            # 1 MB half tiles: tile tt = (t, hh); 8 bufs so the slot-reuse
            # WAW lands on the same DMA sem lane as the FIFO wait (folds).
            sums2_sb = sp.tile([128, 2 * EMB_TILES], F32, tag="sums2")
            for tt in range(2 * EMB_TILES):
                t, hh = tt // 2, tt % 2
                et = embp.tile([128, TW], F32, tag="emb")
                nc.sync.dma_start(
                    et[:], emb[bass.ts(t, 128), bass.ts(hh, TW)]
                )
                nc.vector.reduce_sum(sums2_sb[:, tt : tt + 1], et[:], axis=AX.X)
            # sums_sb[:, t] = sums2[:, 2t] + sums2[:, 2t+1]
            sums_sb = sp.tile([128, EMB_TILES], F32, tag="sums")
            s2v = sums2_sb[:].rearrange("p (t h) -> p h t", h=2)
            nc.vector.tensor_add(sums_sb[:], s2v[:, 0], s2v[:, 1])

            # ---- rearrange sums -> pooled [CIN, (g,b)] ----------------
            # flat channel = b*512 + g*64 + i ; tile t = b*4 + j, g = 2j+q,
            # partition p = q*64 + i.  pooled_sb[i, g*4+b] = sums[...]
            pooled_sb = sp.tile([CIN, G * B_LOC], F32, tag="pooled")
            # dst col = 8j + 4q + b ; src col = 4b + j
            for q in range(2):
                src = sums_sb[q * 64 : (q + 1) * 64, :].rearrange(
                    "i (b j) -> i j b", b=4, j=4
                )
                for j in range(4):
                    nc.gpsimd.dma_start(
                        pooled_sb[:, 8 * j + 4 * q : 8 * j + 4 * q + 4], src[:, j]
                    )

            # ---- phase 2: squeeze MLP (per group) ---------------------
            # stage 1: h[g] = relu(w1[g] @ pooled[g] + b1[g])  (1/HW folded
            # into w1t on host so sums need no explicit mean divide)
            h_sb = sp.tile([128, G * 4 * 4], F32, tag="h")  # col g*16+kc*4+b
            for g in range(G):
                for j in range(4):  # hid chunks of 128
                    ph = psA.tile([128, B_LOC], F32, tag="mm")
                    nc.tensor.matmul(
                        ph[:],
                        w1t_sb[:, g * HID + j * 128 : g * HID + (j + 1) * 128],
                        pooled_sb[:, g * 4 : (g + 1) * 4],
                        start=True,
                        stop=True,
                    )
                    nc.scalar.activation(
                        h_sb[:, g * 16 + j * 4 : g * 16 + j * 4 + 4],
                        ph[:],
                        AF.Relu,
                        bias=b1t_sb[:, g * 4 + j : g * 4 + j + 1],
                    )
            # stage 2: atts[g] = w2[g] @ h[g] + b2[g]
            # atts_sb col = mc*32 + g*4 + b (partition = o2 within chunk mc)
            atts_sb = sp.tile([128, 2 * G * B_LOC], F32, tag="atts")
            for g in range(G):
                for mc in range(2):  # 256 outputs -> 2 chunks of 128
                    pa = psA.tile([128, B_LOC], F32, tag="mm")
                    for kc in range(4):  # K = 512 -> 4 chunks of 128
                        nc.tensor.matmul(
                            pa[:],
                            w2t_sb[
                                :,
                                g * 4 * NCH + kc * NCH + mc * 128 : g * 4 * NCH
                                + kc * NCH
                                + mc * 128
                                + 128,
                            ],
                            h_sb[:, g * 16 + kc * 4 : g * 16 + kc * 4 + 4],
                            start=(kc == 0),
                            stop=(kc == 3),
                        )
                    nc.scalar.activation(
                        atts_sb[:, mc * 32 + g * 4 : mc * 32 + g * 4 + 4],
                        pa[:],
                        AF.Identity,
                        bias=b2t_sb[:, g * 2 + mc : g * 2 + mc + 1],
                    )

            # ---- transpose atts -> xr [ (g,b)=32, (c,o)=256 ] ---------
            xr_sb = sp.tile([32, NCH], F32, tag="xr")
            for mc in range(2):
                pt = psB.tile([32, 128], F32, tag="small")
                nc.tensor.transpose(
                    pt[:], atts_sb[:, mc * 32 : (mc + 1) * 32], i128_sb[:]
                )
                nc.vector.tensor_copy(xr_sb[:, mc * 128 : (mc + 1) * 128], pt[:])

            # ---- phase 3: dynamic routing (3 iters) -------------------
            beta = sp.tile([32, CAPS], F32, tag="beta")
            nc.vector.memset(beta[:], 0.0)
            att_sb = sp.tile([4, NCH], F32, tag="att")
            for it in range(ITERS):
                mx = sp.tile([32, 1], F32, tag="mx")
                nc.vector.reduce_max(mx[:], beta[:], axis=AX.X)
                negmx = sp.tile([32, 1], F32, tag="negmx")
                nc.scalar.mul(negmx[:], mx[:], -1.0)
                e = sp.tile([32, CAPS], F32, tag="e")
                s = sp.tile([32, 1], F32, tag="s")
                nc.scalar.activation(
                    e[:], beta[:], AF.Exp, bias=negmx[:], accum_out=s[:]
                )
                rs = sp.tile([32, 1], F32, tag="rs")
                nc.vector.reciprocal(rs[:], s[:])
                alpha = sp.tile([32, CAPS], F32, tag="alpha")
                nc.vector.tensor_scalar_mul(alpha[:], e[:], rs[:])
                wxr = sp.tile([32, NCH], F32, tag="wxr")
                for c in range(CAPS):
                    nc.vector.tensor_scalar_mul(
                        wxr[:, c * OUT : (c + 1) * OUT],
                        xr_sb[:, c * OUT : (c + 1) * OUT],
                        alpha[:, c : c + 1],
                    )
                vp = psB.tile([4, NCH], F32, tag="small")
                nc.tensor.matmul(vp[:], sel_sb[:], wxr[:], start=True, stop=True)
                if it == ITERS - 1:
                    nc.scalar.activation(att_sb[:], vp[:], AF.Sigmoid)
                else:
                    sq = sp.tile([4, NCH], F32, tag="sq")
                    nc.scalar.square(sq[:], vp[:])
                    n2 = sp.tile([4, CAPS], F32, tag="n2")
                    nc.vector.reduce_sum(
                        n2[:], sq[:].rearrange("p (c o) -> p c o", o=OUT), axis=AX.X
                    )
                    nr = sp.tile([4, CAPS], F32, tag="nr")
                    nc.scalar.sqrt(nr[:], n2[:])
                    nrc = sp.tile([4, CAPS], F32, tag="nrc")
                    nc.vector.tensor_scalar_max(nrc[:], nr[:], 1e-12)
                    rn = sp.tile([4, CAPS], F32, tag="rn")
                    nc.vector.reciprocal(rn[:], nrc[:])
                    vn = sp.tile([4, NCH], F32, tag="vn")
                    for c in range(CAPS):
                        nc.vector.tensor_scalar_mul(
                            vn[:, c * OUT : (c + 1) * OUT],
                            vp[:, c * OUT : (c + 1) * OUT],
                            rn[:, c : c + 1],
                        )
                    bc = psB.tile([32, NCH], F32, tag="small")
                    nc.tensor.matmul(bc[:], selt_sb[:], vn[:], start=True, stop=True)
                    prod = sp.tile([32, NCH], F32, tag="prod")
                    nc.vector.tensor_mul(prod[:], bc[:], xr_sb[:])
                    binc = sp.tile([32, CAPS], F32, tag="binc")
                    nc.vector.reduce_sum(
                        binc[:],
                        prod[:].rearrange("p (c o) -> p c o", o=OUT),
                        axis=AX.X,
                    )
                    nc.vector.tensor_add(beta[:], beta[:], binc[:])

            # ---- transpose att [4, 256] -> attT [128, (ch,b)=8] -------
            attT = sp.tile([128, 2 * B_LOC], F32, tag="attT")
            for ch in range(2):
                pt2 = psB.tile([128, 4], F32, tag="small")
                nc.tensor.transpose(
                    pt2[:], att_sb[:, ch * 128 : (ch + 1) * 128], i4_sb[:]
                )
                nc.vector.tensor_copy(attT[:, ch * 4 : (ch + 1) * 4], pt2[:])

            # ---- phase 4: scale x ------------------------------------
            # x row = b*256 + ch2 ; row block r: b = r//2, ch = r%2
            for tt in range(2 * X_TILES):
                r, hh = tt // 2, tt % 2
                xt = xp.tile([128, TW], F32, tag="x")
                nc.sync.dma_start(xt[:], xin[bass.ts(r, 128), bass.ts(hh, TW)])
                col = (r % 2) * 4 + (r // 2)
                nc.scalar.mul(xt[:], xt[:], attT[:, col : col + 1])
                nc.scalar.dma_start(out[bass.ts(r, 128), bass.ts(hh, TW)], xt[:])

    nc.compile()
    return nc


def _prep_weights(w1, b1, w2, b2):
    w1 = np.asarray(w1, dtype=np.float32)
    b1 = np.asarray(b1, dtype=np.float32)
    w2 = np.asarray(w2, dtype=np.float32)
    b2 = np.asarray(b2, dtype=np.float32)
    # w1t[i, g*512+o] = w1[g, o, i] / HW   (folds the spatial mean)
    w1t = np.ascontiguousarray(
        (w1.transpose(2, 0, 1) / float(HW)).reshape(CIN, G * HID)
    )
    # b1t[p, g*4+j] = b1[g, j*128+p]
    b1t = np.ascontiguousarray(
        b1.reshape(G, 4, 128).transpose(2, 0, 1).reshape(128, G * 4)
    )
    # w2t[p, g*1024 + kc*256 + o2] = w2[g, o2, kc*128+p]
    w2t = np.ascontiguousarray(
        w2.transpose(0, 2, 1)
        .reshape(G, 4, 128, NCH)
        .transpose(2, 0, 1, 3)
        .reshape(128, G * 4 * NCH)
    )
    # b2t[p, g*2+mc] = b2[g, mc*128+p]
    b2t = np.ascontiguousarray(
        b2.reshape(G, 2, 128).transpose(2, 0, 1).reshape(128, G * 2)
    )
    return w1t, b1t, w2t, b2t


def make_in_maps(embedding, x, w1, b1, w2, b2):
    embedding = np.asarray(embedding, dtype=np.float32)
    x = np.asarray(x, dtype=np.float32)
    w1t, b1t, w2t, b2t = _prep_weights(w1, b1, w2, b2)
    in_maps = []
    for c in range(N_CORES):
        in_maps.append(
            {
                "emb": np.ascontiguousarray(
                    embedding[c * B_LOC : (c + 1) * B_LOC]
                ).reshape(EMB_ROWS, HW),
                "xin": np.ascontiguousarray(x[c * B_LOC : (c + 1) * B_LOC]).reshape(
                    X_ROWS, HW
                ),
                "w1t": w1t,
                "b1t": b1t,
                "w2t": w2t,
                "b2t": b2t,
            }
        )
    return in_maps


def kernel(embedding, x, w1, b1, w2, b2):
    nc = build_nc()
    in_maps = make_in_maps(embedding, x, w1, b1, w2, b2)
    res = run_bass_kernel_spmd(nc, in_maps, core_ids=list(range(N_CORES)))
    out = np.concatenate(
        [r["out"].reshape(B_LOC, NCH, 64, 64) for r in res.results], axis=0
    )
    return out
